# revision 84
# baseline (speedup 1.0000x reference)
"""Trainium2 Bass kernel: 16-head causal MHA (B=2, S=2048, hidden=1024).

Sharding (data + head parallel over 8 cores): core c handles batch c//4
and heads [4*(c%4), 4*(c%4)+4). Each core computes its q/k/v projections,
causal attention for its 4 heads, and a partial o-projection restricted to
its head columns. The host sums the 4 partials per batch (the post-o_proj
all-reduce, done host-side during gather) and adds the exactly-linear bias
terms (bv @ wo.T + bo). bq/bk are applied on device via rank-1 bias
matmuls.

Design (all-bf16 matmul pipeline, ~1.45x over the fp32r v1):
  - Host pre-converts x / weights / mask to bf16, so DMA loads land
    directly as matmul operands (no fp32r staging copies) at half the
    HBM traffic. Output partials are written bf16 and summed in fp32 on
    the host. bf16 keeps the end-to-end error ~4e-3 max-rel, inside the
    2e-2 gate. (fp8 was analyzed and rejected: its ~4% quantization
    noise passes through softmax/PV/o_proj essentially undamped.)
  - Layout: qT/kT [d, s]; scoresT/probs [t, s]; v in [t, d] with an
    appended ones column. PV runs TRANSPOSED — the 128-wide probs block
    is the stationary operand and the 65-wide augmented v is moving, so
    each (t-tile, s-block) costs 65 PE cycles instead of 512 per t-tile
    and uses all 128 output partitions. The softmax denominator lands
    as a per-partition column, so normalization is a tiny reciprocal +
    one broadcast-multiply (no PE broadcast matmul); small PE
    transposes (via the eye128 identity) restore [d, s] layout for the
    o-projection.
  - Causal diagonal 512x512 regions are trimmed at 128-column
    granularity in scores, exp, and PV; the two narrow diagonal tiles
    pack into shared PSUM banks (start/stop-chained groups with disjoint
    columns) so exp covers them in single wide ACT calls. The in-block
    triangles are masked with one strided bf16 multiply per diagonal
    tile pair.
  - k-projection runs k-tile-major across all 8 PSUM banks (borrowed
    from the attention pools) so PE consumes each arriving x k-tile in
    3.4us vs its 1.46us DMA: the load phase never starves PE.
  - Engine balance: PE (~100us busy) is the critical resource; exp runs
    on ACT, everything PSUM-touching (projection drains, reciprocal,
    normalization, o-projection drains) on DVE/ACT -- GPSIMD cannot
    access PSUM (BIR verifier rule). The exp stream on ACT paces the
    attention inner loop, so o-projection tiles of the previous chunk,
    leftover projections, and the deferred per-head-pair normalization
    and transposes are spread as PE filler through every head's pair
    loop.
  - The tail o-projection fans out across all 8 freed PSUM banks and
    drains via DVE/ACT with two et-tiles per output DMA.
"""

import numpy as np
import ml_dtypes

import concourse.bass as bass
import concourse.mybir as mybir
import concourse.tile as tile
from concourse import bacc
from concourse.bass_utils import run_bass_kernel_spmd

B, S, HID = 2, 2048, 1024
NH, HD = 16, 64
N_CORES = 8
HPC = 4            # heads per core
DPC = HPC * HD     # 256 head-dims per core
SC = 512           # s-chunk (matmul free dim)
NSC = S // SC      # 4
TT = 128           # t-tile (partitions)
NTT = S // TT      # 16
NKT = HID // 128   # 8 contraction tiles for the projections

F32 = mybir.dt.float32
BF16 = mybir.dt.bfloat16
EXP = mybir.ActivationFunctionType.Exp
BF = ml_dtypes.bfloat16


def _build(causal: bool, has_bias: bool = True):
    nc = bacc.Bacc(
        "TRN2",
        target_bir_lowering=False,
        debug=False,
        enable_asserts=False,
        num_devices=N_CORES,
    )
    xT = nc.dram_tensor("xT", [HID, S], BF16, kind="ExternalInput").ap()
    wqT = nc.dram_tensor("wqT", [HID, DPC], BF16, kind="ExternalInput").ap()
    wkT = nc.dram_tensor("wkT", [HID, DPC], BF16, kind="ExternalInput").ap()
    wvT = nc.dram_tensor("wvT", [HID, DPC], BF16, kind="ExternalInput").ap()
    woT = nc.dram_tensor("woT", [DPC, HID], BF16, kind="ExternalInput").ap()
    bqr = nc.dram_tensor("bq_r", [1, DPC], BF16, kind="ExternalInput").ap()
    bkr = nc.dram_tensor("bk_r", [1, DPC], BF16, kind="ExternalInput").ap()
    mskd = nc.dram_tensor("mask_tri", [TT, TT], BF16, kind="ExternalInput").ap()
    eyed = nc.dram_tensor("eye128", [TT, TT], BF16, kind="ExternalInput").ap()
    outT = nc.dram_tensor("outT", [HID, S], BF16, kind="ExternalOutput").ap()

    ctx_lp = nc.allow_low_precision(reason="bf16 matmul pipeline (deliberate)")
    ctx_lp.__enter__()
    with tile.TileContext(nc) as tc:
        with (
            tc.tile_pool(name="persist", bufs=1) as pp,
            tc.tile_pool(name="expbuf", bufs=6) as e_pool,
            tc.tile_pool(name="attn", bufs=2) as attn_pool,
            tc.tile_pool(name="osb", bufs=6) as o_pool,
            tc.tile_pool(name="small", bufs=3) as sm_pool,
            tc.tile_pool(name="s_ps", bufs=2, space=bass.MemorySpace.PSUM) as s_pool,
            tc.tile_pool(name="pv_ps", bufs=2, space=bass.MemorySpace.PSUM) as pv_pool,
            tc.tile_pool(name="mm_ps", bufs=2, space=bass.MemorySpace.PSUM) as mm_pool,
        ):
            # ---- persistent SBUF tensors (bf16 matmul operands) ----
            x_sb = pp.tile([TT, NKT, S], BF16)     # [k%128, k//128, s]
            wq_sb = pp.tile([TT, NKT, DPC], BF16)
            wk_sb = pp.tile([TT, NKT, DPC], BF16)
            wv_sb = pp.tile([TT, NKT, DPC], BF16)
            wo_sb = pp.tile([TT, 2, HID], BF16)
            qT_sb = pp.tile([TT, 2, S], BF16)      # [d%128, d//128, s]
            kT_sb = pp.tile([TT, 2, S], BF16)
            v_sb = pp.tile([TT, NTT, HPC, HD + 1], BF16)  # [t%128, t//128, h, d|1]
            ones_sb = pp.tile([1, SC], BF16)
            mask_sb = pp.tile([TT, TT], BF16)
            eye_sb = pp.tile([TT, TT], BF16)
            bq_sb = pp.tile([1, DPC], BF16)
            bk_sb = pp.tile([1, DPC], BF16)

            nc.vector.memset(ones_sb[:], 1.0)
            # ones columns of the augmented v (softmax denominator trick)
            nc.vector.memset(v_sb[:, :, :, HD : HD + 1], 1.0)

            # ---- loads: weights/x stream straight into bf16 SBUF ----
            # All loads go through the (serialized) DMA path: wk first,
            # then full-row x k-tiles. The k-major projection phase below
            # consumes each x k-tile in 3.4us of PE work vs its 1.46us
            # DMA, so PE streams continuously from the first tile on.
            def load_w(dst, src):
                nc.sync.dma_start(
                    out=dst[:],
                    in_=src.rearrange("(k p) c -> p k c", p=TT),
                )

            # wk k-slices interleave with the x k-tiles so each k-group's
            # stationary arrives just before its x tile (the 182ns wk
            # slices hide inside the 1456ns x transfers)
            for k in range(NKT):
                nc.sync.dma_start(
                    out=wk_sb[:, k, :],
                    in_=wkT[128 * k : 128 * (k + 1), :],
                )
                nc.sync.dma_start(
                    out=x_sb[:, k, :],
                    in_=xT[128 * k : 128 * (k + 1), :],
                )
            load_w(wq_sb, wqT)
            load_w(wv_sb, wvT)
            nc.sync.dma_start(
                out=wo_sb[:],
                in_=woT.rearrange("(k p) c -> p k c", p=TT),
            )
            nc.sync.dma_start(out=mask_sb[:], in_=mskd[:])
            nc.sync.dma_start(out=eye_sb[:], in_=eyed[:])
            if has_bias:
                nc.sync.dma_start(out=bq_sb[:], in_=bqr[:])
                nc.sync.dma_start(out=bk_sb[:], in_=bkr[:])

            # ---- projection emission helpers ----
            # GPSIMD cannot touch PSUM (BIR verifier rule), so every
            # PSUM->SBUF drain runs on DVE or ACT; phase-A copies use the
            # then-idle ACT, attention-phase copies use DVE.
            def ps_copy(eng, out, in_):
                if eng is nc.scalar:
                    nc.scalar.copy(out, in_)
                else:
                    eng.tensor_copy(out, in_)
            def proj_qk(w_sb, b_sb, dst, dti, sc, eng=None):
                q_ps = mm_pool.tile([TT, SC], F32, tag="mm")
                for k in range(NKT):
                    nc.tensor.matmul(
                        q_ps[:],
                        w_sb[:, k, 128 * dti : 128 * (dti + 1)],
                        x_sb[:, k, SC * sc : SC * (sc + 1)],
                        start=(k == 0),
                        stop=(k == NKT - 1 and not has_bias),
                    )
                if has_bias:
                    nc.tensor.matmul(
                        q_ps[:],
                        b_sb[0:1, 128 * dti : 128 * (dti + 1)],
                        ones_sb[0:1, :],
                        start=False,
                        stop=True,
                    )
                ps_copy(eng or nc.vector, dst[:, dti, SC * sc : SC * (sc + 1)], q_ps[:])

            def proj_k_all():
                # All 8 (dti, sc) k-projection groups accumulate at once,
                # k-tile-major, across every PSUM bank (borrowed from the
                # attention pools, which are not yet in use): each arriving
                # x k-tile immediately feeds 16 matmuls, so the phase-A
                # critical path is the x DMA stream, not PE stalls.
                tA = s_pool.tile([TT, 2, SC], F32, name="kgA", tag="s")
                tB = s_pool.tile([TT, 2, SC], F32, name="kgB", tag="s")
                tC = pv_pool.tile([TT, SC], F32, name="kgC", tag="pv")
                tD = pv_pool.tile([TT, SC], F32, name="kgD", tag="pv")
                tE = mm_pool.tile([TT, SC], F32, name="kgE", tag="mm")
                tF = mm_pool.tile([TT, SC], F32, name="kgF", tag="mm")

                def grp(dti, sc):
                    if dti == 0:
                        return (tA if sc < 2 else tB)[:, sc % 2, :]
                    return [tC, tD, tE, tF][sc][:]

                # group order: the mm/pv-pool groups complete first within
                # k=7, so their banks free earliest for the q/v projection
                # allocations that WAR on them
                order = [(1, 2), (1, 3), (0, 0), (0, 1), (0, 2), (0, 3),
                         (1, 0), (1, 1)]
                for k in range(NKT):
                    for dti, sc in order:
                        nc.tensor.matmul(
                            grp(dti, sc),
                            wk_sb[:, k, 128 * dti : 128 * (dti + 1)],
                            x_sb[:, k, SC * sc : SC * (sc + 1)],
                            start=(k == 0),
                            stop=(k == NKT - 1 and not has_bias),
                        )
                # copies alternate DVE/Pool so the 8 drains run in parallel
                for n, (dti, sc) in enumerate(order):
                    if has_bias:
                        nc.tensor.matmul(
                            grp(dti, sc),
                            bk_sb[0:1, 128 * dti : 128 * (dti + 1)],
                            ones_sb[0:1, :],
                            start=False,
                            stop=True,
                        )
                    ps_copy(
                        nc.scalar if n % 2 == 0 else nc.vector,
                        kT_sb[:, dti, SC * sc : SC * (sc + 1)],
                        grp(dti, sc),
                    )

            def proj_v(tt, eng=None):
                v_ps = mm_pool.tile([TT, DPC], F32, tag="mm")
                for k in range(NKT):
                    nc.tensor.matmul(
                        v_ps[:],
                        x_sb[:, k, 128 * tt : 128 * (tt + 1)],
                        wv_sb[:, k, :],
                        start=(k == 0),
                        stop=(k == NKT - 1),
                    )
                ps_copy(
                    eng or nc.vector,
                    v_sb[:, tt, :, 0:HD],
                    v_ps[:].rearrange("p (h d) -> p h d", h=HPC),
                )

            def attn_head(sc, h, attn_sb, attn_T, mid=None):
                dti, po = h // 2, 64 * (h % 2)
                n_tt = 4 * (sc + 1) if causal else NTT
                n_full = n_tt - 4 if causal else NTT
                # pv accumulates TRANSPOSED: out[s-block, d] with the
                # 128-wide probs block as the stationary operand and the
                # 65-wide v as moving — 65 PE cycles per (t-tile, s-block)
                # instead of 512 per t-tile, full 128-partition output,
                # and the softmax denominator lands per-partition (col 64)
                # so normalization needs no broadcast matmul.
                pv_ps = pv_pool.tile([TT, 4, HD + 1], F32, tag="pv")
                scale = float(1.0 / np.sqrt(HD))
                last_tt = n_tt - 1
                bank_virgin = [True]

                def score_mm(out_ap, tt, c0, start=True, stop=True, skip=False):
                    nc.tensor.matmul(
                        out_ap,
                        kT_sb[po : po + 64, dti, 128 * tt : 128 * (tt + 1)],
                        qT_sb[po : po + 64, dti, SC * sc + c0 : SC * (sc + 1)],
                        start=start,
                        stop=stop,
                        skip_group_check=skip,
                    )

                def pv_mm(tt, blocks):
                    # blocks: list of (s_subtile_j, probs_block_ap)
                    for j, ap in blocks:
                        stop = (tt == n_full + j) if causal else (tt == last_tt)
                        nc.tensor.matmul(
                            pv_ps[:, j, :],
                            ap,
                            v_sb[:, tt, h, :],
                            start=bank_virgin[0],
                            stop=stop,
                            skip_group_check=True,
                        )
                        bank_virgin[0] = False

                def mask_mul(e_ap):
                    nc.vector.tensor_mul(e_ap, e_ap, mask_sb[:])

                # Each "pair" = a scores+exp emitter plus its deferred PV
                # emitter: PV(pair j) runs after scores(pair j+1) so PE is
                # not exposed to the exp latency; `mid` fillers pop every
                # other pair to cover ACT's slightly higher per-pair cost.
                def full_pair(t0):
                    def scores():
                        s_ps = s_pool.tile([TT, 2, SC], F32, tag="s")
                        for i in range(2):
                            score_mm(s_ps[:, i, :], t0 + i, 0)
                        e_sb = e_pool.tile([TT, 2, SC], BF16, tag="e")
                        nc.scalar.activation(e_sb[:], s_ps[:], EXP, scale=scale)
                        return e_sb

                    def pvs(e_sb):
                        pv_mm(t0, [(j, e_sb[:, 0, 128 * j : 128 * (j + 1)])
                                   for j in range(4)])
                        pv_mm(t0 + 1, [(j, e_sb[:, 1, 128 * j : 128 * (j + 1)])
                                       for j in range(4)])

                    return scores, pvs

                def diag_pair_a():
                    # diagonal tiles r0 (512 cols) and r1 (384 cols, packed
                    # at plane-1 col 0 so one exp call covers both)
                    r0, r1 = n_full, n_full + 1

                    def scores():
                        s_ps = s_pool.tile([TT, 2, SC], F32, tag="s")
                        score_mm(s_ps[:, 0, :], r0, 0)
                        score_mm(s_ps[:, 1, 0:384], r1, 128)
                        e_sb = e_pool.tile([TT, 2, SC], BF16, tag="e")
                        nc.scalar.activation(
                            e_sb.rearrange("p a b -> p (a b)")[:, 0:896],
                            s_ps.rearrange("p a b -> p (a b)")[:, 0:896],
                            EXP,
                            scale=scale,
                        )
                        m2 = e_sb[:, 0:2, 0:128]
                        nc.vector.tensor_mul(
                            m2, m2, mask_sb[:].rearrange("p (a b) -> p a b", a=1).broadcast_to((TT, 2, TT))
                        )
                        return e_sb

                    def pvs(e_sb):
                        # r1's probs are packed at plane-1 col 0 (s-cols
                        # 128:512): s-block j lives at cols 128(j-1):128j
                        pv_mm(r0, [(j, e_sb[:, 0, 128 * j : 128 * (j + 1)])
                                   for j in range(4)])
                        pv_mm(r1, [(j, e_sb[:, 1, 128 * (j - 1) : 128 * j])
                                   for j in range(1, 4)])

                    return scores, pvs

                def diag_pair_b():
                    # diagonal tiles r2 (256 cols) and r3 (128 cols) packed
                    # into one PSUM bank as a single start/stop group with
                    # disjoint column ranges; one exp call covers both
                    r2, r3 = n_full + 2, n_full + 3

                    def scores():
                        s_ps = s_pool.tile([TT, 2, SC], F32, tag="s")
                        score_mm(s_ps[:, 0, 0:256], r2, 256,
                                 start=True, stop=False, skip=True)
                        score_mm(s_ps[:, 0, 256:384], r3, 384,
                                 start=False, stop=True, skip=True)
                        e_sb = e_pool.tile([TT, 2, SC], BF16, tag="e")
                        nc.scalar.activation(
                            e_sb[:, 0, 0:384], s_ps[:, 0, 0:384], EXP,
                            scale=scale,
                        )
                        m2 = e_sb[:, 0, 0:384].rearrange(
                            "p (a b) -> p a b", b=TT
                        )[:, ::2, :]
                        nc.vector.tensor_mul(
                            m2, m2, mask_sb[:].rearrange("p (a b) -> p a b", a=1).broadcast_to((TT, 2, TT))
                        )
                        return e_sb

                    def pvs(e_sb):
                        # r2 packed at plane-0 cols 0:256 (s 256:512),
                        # r3 at plane-0 cols 256:384 (s 384:512)
                        pv_mm(r2, [(j, e_sb[:, 0, 128 * (j - 2) : 128 * (j - 1)])
                                   for j in (2, 3)])
                        pv_mm(r3, [(3, e_sb[:, 0, 256:384])])

                    return scores, pvs

                pairs = [full_pair(t0) for t0 in range(0, n_full, 2)]
                if causal:
                    pairs.append(diag_pair_a())
                    pairs.append(diag_pair_b())
                mid = list(mid) if mid else []
                prev_pv, prev_e = None, None
                for j, (scores_fn, pvs_fn) in enumerate(pairs):
                    e_sb = scores_fn()
                    if j >= 1 and mid:
                        mid.pop(0)()
                    if prev_pv is not None:
                        prev_pv(prev_e)
                    prev_pv, prev_e = pvs_fn, e_sb
                prev_pv(prev_e)

                # the denominators sit in pv col 64 as a per-partition
                # column: a tiny reciprocal replaces the old wide
                # reciprocal + PE broadcast + staging copy
                rc_sb = sm_pool.tile([TT, 4, 1], F32, tag="rc")
                nc.vector.reciprocal(rc_sb[:], pv_ps[:, :, HD : HD + 1])
                # leftover filler the pair loop didn't consume
                for f in mid:
                    f()

                def finalize():
                    nc.vector.tensor_mul(
                        attn_T[:, :, 64 * h : 64 * (h + 1)],
                        pv_ps[:, :, 0:HD],
                        rc_sb[:].broadcast_to((TT, 4, HD)),
                    )
                    if h % 2 == 1:
                        # both heads of this dti are normalized: transpose
                        # the four [128s, 128d] blocks back to [d, s]
                        # layout for the o-projection
                        for j in range(4):
                            tp = mm_pool.tile(
                                [TT, TT], BF16, tag="mm", name="tp"
                            )
                            nc.tensor.transpose(
                                tp[:],
                                attn_T[:, j, 128 * dti : 128 * (dti + 1)],
                                eye_sb[:],
                            )
                            nc.vector.tensor_copy(
                                attn_sb[:, dti, 128 * j : 128 * (j + 1)],
                                tp[:],
                            )

                return finalize

            def oproj_et(sc, attn_sb, et, split=False):
                o_ps = mm_pool.tile([TT, SC], F32, tag="mm")
                for dti in range(2):
                    nc.tensor.matmul(
                        o_ps[:],
                        wo_sb[:, dti, 128 * et : 128 * (et + 1)],
                        attn_sb[:, dti, :],
                        start=(dti == 0),
                        stop=(dti == 1),
                    )
                o_sb = o_pool.tile([TT, SC], BF16)
                # mid-kernel o-drains stay off ACT (the exp stream is the
                # near-critical consumer there); DVE has headroom
                ps_copy(nc.vector, o_sb[:], o_ps[:])
                nc.sync.dma_start(
                    out=outT[128 * et : 128 * (et + 1),
                             SC * sc : SC * (sc + 1)],
                    in_=o_sb[:],
                )

            # ---- phase A: projections needed before attention can start:
            # all of k (k-major across every PSUM bank), q chunk 0, and
            # v t-tiles 0-3.
            proj_k_all()
            proj_qk(wq_sb, bq_sb, qT_sb, 0, 0, eng=nc.scalar)
            proj_qk(wq_sb, bq_sb, qT_sb, 1, 0)
            for i in range(4):
                proj_v(i, eng=(nc.scalar if i % 2 == 0 else nc.vector))

            # ---- phase B: attention. Remaining projections and the
            # previous chunk's o-projection interleave as PE filler after
            # each head (the exp stream on ACT slightly outpaces the
            # scores+PV stream on PE, so PE needs background work).
            units = [
                [
                    [lambda: proj_v(4)],
                    [lambda: proj_v(5)],
                    [lambda: proj_qk(wq_sb, bq_sb, qT_sb, 0, 1), lambda: proj_v(6)],
                    [lambda: proj_qk(wq_sb, bq_sb, qT_sb, 1, 1), lambda: proj_v(7)],
                ],
                [
                    [lambda: proj_v(8)],
                    [lambda: proj_v(9)],
                    [lambda: proj_qk(wq_sb, bq_sb, qT_sb, 0, 2), lambda: proj_v(10)],
                    [lambda: proj_qk(wq_sb, bq_sb, qT_sb, 1, 2), lambda: proj_v(11)],
                ],
                [
                    [lambda: proj_qk(wq_sb, bq_sb, qT_sb, 0, 3)],
                    [lambda: proj_qk(wq_sb, bq_sb, qT_sb, 1, 3), lambda: proj_v(12)],
                    [lambda: proj_v(13), lambda: proj_v(14)],
                    [lambda: proj_v(15)],
                ],
                [[], [], [], []],
            ]
            pending = None
            prev = None  # (sc, attn_sb) of the chunk awaiting o-projection
            for sc in range(NSC):
                attn_sb = attn_pool.tile([TT, 2, SC], BF16)
                attn_T = attn_pool.tile([TT, 4, DPC], BF16, tag="aT", name="attn_T")
                for h in range(HPC):
                    # PE filler for this head's pair loop: the previous
                    # head's deferred normalization first (it must precede
                    # any o-proj read of its attn_sb), then two tiles of
                    # the previous chunk's o-projection, then projections
                    fillers = []
                    if prev is not None:
                        p_sc, p_attn = prev
                        for et in (2 * h, 2 * h + 1):
                            fillers.append(
                                lambda p_sc=p_sc, p_attn=p_attn, et=et: oproj_et(
                                    p_sc, p_attn, et
                                )
                            )
                    if pending is not None:
                        # at a chunk boundary (h==0) the finalize must
                        # precede the o-proj reads of its chunk; elsewhere
                        # it can trail one filler to give the reciprocal
                        # more slack
                        fillers.insert(0 if h == 0 else 2, pending)
                    fillers.extend(units[sc][h])
                    pending = attn_head(sc, h, attn_sb, attn_T, mid=fillers)
                prev = (sc, attn_sb)
            pending()
            # tail o-projection: attention is done, so every PSUM bank is
            # free — run all 8 et-tiles' matmuls back to back into 8 banks,
            # then drain them in parallel on both copy engines
            s1 = s_pool.tile([TT, 2, SC], F32, name="ot_s1", tag="s")
            s2 = s_pool.tile([TT, 2, SC], F32, name="ot_s2", tag="s")
            p1 = pv_pool.tile([TT, SC], F32, name="ot_p1", tag="pv")
            p2 = pv_pool.tile([TT, SC], F32, name="ot_p2", tag="pv")
            m1 = mm_pool.tile([TT, SC], F32, name="ot_m1", tag="mm")
            m2 = mm_pool.tile([TT, SC], F32, name="ot_m2", tag="mm")
            banks = [s1[:, 0, :], s1[:, 1, :], s2[:, 0, :], s2[:, 1, :],
                     p1[:], p2[:], m1[:], m2[:]]
            attn_sb = prev[1]
            o_big = None
            for et in range(NKT):
                for dti in range(2):
                    nc.tensor.matmul(
                        banks[et],
                        wo_sb[:, dti, 128 * et : 128 * (et + 1)],
                        attn_sb[:, dti, :],
                        start=(dti == 0),
                        stop=(dti == 1),
                    )
                # drain rotates over Pool/DVE/ACT (all idle at the tail);
                # two et-tiles share one DMA to halve the serial HWDGE cost
                if et % 2 == 0:
                    o_big = o_pool.tile([TT, 2, SC], BF16, name="ot_o", tag="o")
                ps_copy(
                    nc.vector if et % 2 == 0 else nc.scalar,
                    o_big[:, et % 2, :],
                    banks[et][:],
                )
                if et % 2 == 1:
                    nc.sync.dma_start(
                        out=outT[
                            128 * (et - 1) : 128 * (et + 1), SC * (NSC - 1) : S
                        ].rearrange("(k p) c -> p k c", p=TT),
                        in_=o_big[:],
                    )
    ctx_lp.__exit__(None, None, None)
    nc.compile()
    return nc


_CACHE = {}
LAST_RESULTS = None


def _get_nc(causal: bool, has_bias: bool = False):
    key = (causal, has_bias)
    if key not in _CACHE:
        _CACHE[key] = _build(causal, has_bias)
    return _CACHE[key]


def _reference_host(hidden_state, attention_mask, wq, bq, wk, bk, wv, bv, wo, bo):
    """Exact numpy fallback for unexpected mask patterns."""
    x = hidden_state.astype(np.float64)
    q = (x @ wq.T.astype(np.float64) + bq).reshape(B, S, NH, HD).transpose(0, 2, 1, 3)
    k = (x @ wk.T.astype(np.float64) + bk).reshape(B, S, NH, HD).transpose(0, 2, 1, 3)
    v = (x @ wv.T.astype(np.float64) + bv).reshape(B, S, NH, HD).transpose(0, 2, 1, 3)
    sc = np.einsum("bhsd,bhtd->bhst", q, k) / np.sqrt(HD)
    sc = np.where(attention_mask, sc, -np.inf)
    sc -= sc.max(axis=-1, keepdims=True)
    e = np.exp(sc)
    p = e / e.sum(axis=-1, keepdims=True)
    o = np.einsum("bhst,bhtd->bhsd", p, v).transpose(0, 2, 1, 3).reshape(B, S, HID)
    return (o @ wo.T.astype(np.float64) + bo).astype(np.float32)


def kernel(hidden_state, attention_mask, wq, bq, wk, bk, wv, bv, wo, bo):
    global LAST_RESULTS
    hidden_state = np.asarray(hidden_state, dtype=np.float32)
    attention_mask = np.asarray(attention_mask, dtype=bool)
    wq, bq = np.asarray(wq, np.float32), np.asarray(bq, np.float32)
    wk, bk = np.asarray(wk, np.float32), np.asarray(bk, np.float32)
    wv, bv = np.asarray(wv, np.float32), np.asarray(bv, np.float32)
    wo, bo = np.asarray(wo, np.float32), np.asarray(bo, np.float32)

    tril = np.tril(np.ones((S, S), dtype=bool))
    if (attention_mask == tril).all():
        causal = True
    elif attention_mask.all():
        causal = False
    else:
        return _reference_host(
            hidden_state, attention_mask, wq, bq, wk, bk, wv, bv, wo, bo
        )

    mask_tri = np.triu(np.ones((TT, TT), dtype=BF))
    eye128 = np.eye(TT, dtype=BF)
    in_maps = []
    for c in range(N_CORES):
        b, g = c // 4, c % 4
        r0 = DPC * g
        in_maps.append(
            {
                "xT": np.ascontiguousarray(hidden_state[b].T.astype(BF)),
                "wqT": np.ascontiguousarray(wq[r0 : r0 + DPC].T.astype(BF)),
                "wkT": np.ascontiguousarray(wk[r0 : r0 + DPC].T.astype(BF)),
                "wvT": np.ascontiguousarray(wv[r0 : r0 + DPC].T.astype(BF)),
                "woT": np.ascontiguousarray(wo[:, r0 : r0 + DPC].T.astype(BF)),
                "bq_r": np.ascontiguousarray(
                    bq[r0 : r0 + DPC].reshape(1, DPC).astype(BF)
                ),
                "bk_r": np.ascontiguousarray(
                    bk[r0 : r0 + DPC].reshape(1, DPC).astype(BF)
                ),
                "mask_tri": mask_tri,
                "eye128": eye128,
            }
        )

    has_bias = bool(np.any(bq) or np.any(bk))
    nc = _get_nc(causal, has_bias)
    res = run_bass_kernel_spmd(nc, in_maps, list(range(N_CORES)))
    LAST_RESULTS = res

    out = np.zeros((B, S, HID), dtype=np.float32)
    for c in range(N_CORES):
        out[c // 4] += res.results[c]["outT"].T.astype(np.float32)
    out += (bv @ wo.T + bo)[None, None, :]
    return out
